# revision 1
# baseline (speedup 1.0000x reference)
"""DTCWT inverse (qshift, single level) as a Bass/Tile kernel for TRN2.

Per-core computation, per (channel) slice:  Y = Ccat @ Xcat @ Rcat
with Xcat = [[Yl, hl], [lh, hh]] (c2q quadrants), Ccat/Rcat static banded
synthesis matrices. Implemented as two matmul stages that both consume
natural-layout data as the stationary operand:
    Tt = Xcat^T @ Ccat^T   (mm1, data stationary, statics moving)
    Y  = Tt^T  @ Rcat      (mm2)
Foldings (all host-side, into the static matrices):
  - quadrant row order:  D_E rows = [even; odd] (rho), D_O rows = [odd; even]
  - column order pi = [even cols; odd cols] per 128-block  -> Rcat rows
  - c2q 1/sqrt(2) scale -> quadrant statics
c2q itself is 2 scalar_tensor_tensor ops per quadrant (per-partition sign
vector), all operands partition-aligned at 0.
"""
import numpy as np

import concourse.bacc as bacc
import concourse.tile as tile
from concourse import mybir

F32 = mybir.dt.float32
F32R = mybir.dt.float32r

# ---------------- host-side static matrix construction ----------------

_H0A = np.array([0.0351638365171441, 0.0, -0.0883294244510729,
                 0.233890320607236, 0.760272369066126, 0.587518297723561,
                 0.0, -0.114301837144249, 0.0, 0.0], dtype=np.float64)
_H0B = _H0A[::-1].copy()
_ALT = (-1.0) ** np.arange(10)
_H1A = _H0B * _ALT
_H1B = _H1A[::-1].copy()
G0A, G0B, G1A, G1B = _H0B, _H0A, _H1B, _H1A

RHO_E = np.concatenate([np.arange(0, 128, 2), np.arange(1, 128, 2)])  # [even;odd]
RHO_O = np.concatenate([np.arange(1, 128, 2), np.arange(0, 128, 2)])  # [odd;even]
PI = RHO_E  # column order: even cols first


def _reflect(x, minx, maxx):
    x = np.asarray(x, dtype=np.float64)
    rng = maxx - minx
    rng2 = 2.0 * rng
    mod = np.fmod(x - minx, rng2)
    normed = np.where(mod < 0, mod + rng2, mod)
    return (np.where(normed >= rng, rng2 - normed, normed) + minx).astype(np.int64)


def _colifilt_matrix(ha, hb, r=128):
    """C (2r x r) with colifilt(X) = C @ X."""
    m = ha.shape[0]
    m2 = m // 2
    xe = _reflect(np.arange(-m2, r + m2), -0.5, r - 0.5)
    t = np.arange(2, r + m - 1, 2)
    if float(np.sum(ha * hb)) > 0:
        ta, tb = t, t - 1
    else:
        ta, tb = t - 1, t
    r2 = r // 2
    hao, hae = ha[0::2], ha[1::2]
    hbo, hbe = hb[0::2], hb[1::2]

    def vconv_mat(sel_idx, h):
        hf = h[::-1]
        M = np.zeros((r2, r), dtype=np.float64)
        for i in range(r2):
            for k in range(m2):
                M[i, sel_idx[i + k]] += hf[k]
        return M

    C = np.zeros((2 * r, r), dtype=np.float64)
    C[0::4] = vconv_mat(xe[tb], hao)
    C[1::4] = vconv_mat(xe[ta], hbo)
    C[2::4] = vconv_mat(xe[tb], hae)
    C[3::4] = vconv_mat(xe[ta], hbe)
    return C


def build_statics():
    """STAT1 (128 x 1280) = [S_TL | S_TR_E | S_TR_O | S_BL_E | S_BL_O
                             | ... wait: packed as 5 blocks? see below]
    Layout: [S_TL (256) | S_C0_E (256) | S_C0_O (256) | S_C1_E (256) | S_C1_O (256)]
      S_TL   = C0^T (natural rows)                       -- for the TL matmul
      S_C0_E = s * C0^T rows rho_E                       -- TR (hl) even cols
      S_C0_O = s * C0^T rows rho_O                       -- TR odd cols
      S_C1_E = s * C1^T rows rho_E                       -- BL (lh) / BR (hh) even
      S_C1_O = s * C1^T rows rho_O                       -- BL / BR odd
    STAT2 (128 x 512) = [R_lo' | R_hi'] with rows pi-permuted.
    SIGNS (128 x 2): col0 = [+1]*64+[-1]*64, col1 = [-1]*64+[+1]*64.
    """
    C0 = _colifilt_matrix(G0B, G0A)
    C1 = _colifilt_matrix(G1B, G1A)
    s = 1.0 / np.sqrt(2.0)
    # partition p of a band tile holds row r=p//2 of (real if p even else
    # imag); D_E row semantics are then the natural quadrant rows, D_O rows
    # are pair-swapped.
    swap = np.arange(128) ^ 1
    S_TL = C0.T
    S_C0_E = (s * C0).T
    S_C0_O = (s * C0[:, swap]).T
    S_C1_E = (s * C1).T
    S_C1_O = (s * C1[:, swap]).T
    STAT1 = np.concatenate([S_TL, S_C0_E, S_C0_O, S_C1_E, S_C1_O],
                           axis=1).astype(np.float32)
    R_lo = C0.T[PI]   # rows = Xcat cols, pi-permuted
    R_hi = C1.T[PI]
    STAT2 = np.concatenate([R_lo, R_hi], axis=1).astype(np.float32)
    SIGNS = np.zeros((128, 2), dtype=np.float32)
    SIGNS[0::2, 0] = 1.0    # x1 = +w2r + w1r   (even p = real rows)
    SIGNS[1::2, 0] = -1.0   # x3 = -w2i + w1i   (odd p = imag rows)
    SIGNS[0::2, 1] = -1.0   # x4 = -w1r + w2r
    SIGNS[1::2, 1] = 1.0    # x2 = +w1i + w2i
    return (np.ascontiguousarray(STAT1), np.ascontiguousarray(STAT2),
            np.ascontiguousarray(SIGNS))


# ---------------- device kernel ----------------

QUADS = [("hl", 2, 3, "C0"), ("lh", 0, 5, "C1"), ("hh", 1, 4, "C1")]


def build_kernel(n_ch=64, G=8, n_cores=8, merged_tl=False, debug_taps=False):
    """Build the per-core Bass module. Each core processes n_ch slices."""
    nc = bacc.Bacc("TRN2", target_bir_lowering=False, debug=False,
                   num_devices=n_cores)
    Yl = nc.dram_tensor("Yl", [n_ch, 128, 128], F32R, kind="ExternalInput").ap()
    YH = nc.dram_tensor("YH", [n_ch, 6, 128, 64], F32R, kind="ExternalInput").ap()
    ST1 = nc.dram_tensor("STAT1", [128, 1280], F32R, kind="ExternalInput").ap()
    ST2 = nc.dram_tensor("STAT2", [128, 512], F32R, kind="ExternalInput").ap()
    SGN = nc.dram_tensor("SIGNS", [128, 2], F32R, kind="ExternalInput").ap()
    OUT = nc.dram_tensor("Y", [n_ch, 256, 256], F32, kind="ExternalOutput").ap()

    assert n_ch % G == 0
    with tile.TileContext(nc) as tc:
        with (
            tc.tile_pool(name="const", bufs=1) as const,
            tc.tile_pool(name="inp", bufs=2) as inp,
            tc.tile_pool(name="quad", bufs=2) as quad,
            tc.tile_pool(name="tt", bufs=3) as ttp,
            tc.tile_pool(name="yout", bufs=2) as yp,
            tc.tile_pool(name="psum", bufs=3, space="PSUM") as pp,
            tc.tile_pool(name="psumy", bufs=2, space="PSUM") as ppy,
        ):
            s1 = const.tile([128, 1280], F32R)
            nc.sync.dma_start(s1[:], ST1[:])
            s2 = const.tile([128, 512], F32R)
            nc.sync.dma_start(s2[:], ST2[:])
            sgn = const.tile([128, 2], F32R)
            nc.sync.dma_start(sgn[:], SGN[:])

            # static rhs blocks
            S_TL = s1[:, 0:256]
            S_E = {"C0": s1[:, 256:512], "C1": s1[:, 768:1024]}
            S_O = {"C0": s1[:, 512:768], "C1": s1[:, 1024:1280]}
            R_lo = s2[:, 0:256]
            R_hi = s2[:, 256:512]
            s_a = sgn[:, 0:1]
            s_b = sgn[:, 1:2]


            def load_group(g0):
                TL = inp.tile([128, 128 * G], F32R, tag="TL")
                nc.sync.dma_start(
                    TL.rearrange("p (g c) -> p g c", g=G),
                    Yl[g0:g0 + G].rearrange("g p c -> p g c"),
                )
                bts = {}
                for qname, b1, b2, cmat in QUADS:
                    bt = inp.tile([128, 128 * G], F32R, tag=f"bt_{qname}")
                    btv = bt.rearrange("p (g b c) -> p g b c", g=G, b=2)
                    for bi, b in ((0, b1), (1, b2)):
                        nc.sync.dma_start(
                            btv[:, :, bi],
                            YH[g0:g0 + G, b].rearrange("g p c -> p g c"),
                        )
                    bts[qname] = btv
                return TL, bts

            def prep_group(state):
                TL, bts = state
                # odd Yl columns, packed contiguous (even cols are read from
                # TL with an even-offset stride-2 weight AP, which is legal)
                TL_O = inp.tile([128, 64 * G], F32R, tag="TL_O")
                nc.gpsimd.tensor_copy(
                    TL_O.rearrange("p (g j) -> p g j", g=G),
                    TL.rearrange("p (g j two) -> p g j two", g=G, two=2)[:, :, :, 1],
                )
                qt = {}
                for qname, b1, b2, cmat in QUADS:
                    btv = bts[qname]
                    B1 = btv[:, :, 0]   # rows: [w1r/w1i interleaved]
                    B2 = btv[:, :, 1]
                    D_E = quad.tile([128, 64 * G], F32R, tag=f"q_{qname}_E")
                    D_O = quad.tile([128, 64 * G], F32R, tag=f"q_{qname}_O")
                    qt[qname] = (D_E, D_O)
                    dev = D_E.rearrange("p (g c) -> p g c", g=G)
                    dov = D_O.rearrange("p (g c) -> p g c", g=G)
                    # D_E: even p: x1 = w2r + w1r ; odd p: x3 = -w2i + w1i
                    nc.vector.scalar_tensor_tensor(
                        dev, B2, s_a, B1,
                        op0=mybir.AluOpType.mult, op1=mybir.AluOpType.add)
                    # D_O: even p: x4 = -w1r + w2r ; odd p: x2 = w1i + w2i
                    nc.vector.scalar_tensor_tensor(
                        dov, B1, s_b, B2,
                        op0=mybir.AluOpType.mult, op1=mybir.AluOpType.add)
                return TL, TL_O, qt
            def process_group(g0, state, mid_emit=None):
                TL, TL_O, qt = state
                YB = yp.tile([128, 512 * G], F32, tag="yb")
                for ci in range(G):
                    if ci == 3 and mid_emit is not None:
                        mid_emit()
                    qs = slice(ci * 64, (ci + 1) * 64)
                    # fp32r matmuls cannot target PSUM partition base 64, so
                    # E/O halves go to free-dim halves of a 64p region; one
                    # two-bank tile per slice (bank0 = tt0, bank1 = tt1).
                    ttf = pp.tile([128, 1024], F32, tag="ttb")
                    tt0 = ttf[0:64, 0:512]
                    tt1 = ttf[0:64, 512:1024]
                    tl_even = TL[:, ci * 128:(ci + 1) * 128].rearrange(
                        "p (j two) -> p j two", two=2)[:, :, 0]
                    tl_odd = TL_O[:, qs]
                    # ONE start=True per PSUM bank: start marks the whole
                    # bank pending-zero, later matmuls accumulate anywhere.
                    nc.tensor.matmul(tt0[:, 0:256], tl_even, S_TL,
                                     start=True, stop=False, skip_group_check=True)
                    nc.tensor.matmul(tt0[:, 256:512], tl_odd, S_TL,
                                     start=False, stop=False, skip_group_check=True)
                    lhE, lhO = qt["lh"]
                    nc.tensor.matmul(tt0[:, 0:256], lhE[:, qs], S_E["C1"],
                                     start=False, stop=False, skip_group_check=True)
                    nc.tensor.matmul(tt0[:, 256:512], lhO[:, qs], S_O["C1"],
                                     start=False, stop=True, skip_group_check=True)
                    hlE, hlO = qt["hl"]
                    hhE, hhO = qt["hh"]
                    nc.tensor.matmul(tt1[:, 0:256], hlE[:, qs], S_E["C0"],
                                     start=True, stop=False, skip_group_check=True)
                    nc.tensor.matmul(tt1[:, 0:256], hhE[:, qs], S_E["C1"],
                                     start=False, stop=False, skip_group_check=True)
                    nc.tensor.matmul(tt1[:, 256:512], hlO[:, qs], S_O["C0"],
                                     start=False, stop=False, skip_group_check=True)
                    nc.tensor.matmul(tt1[:, 256:512], hhO[:, qs], S_O["C1"],
                                     start=False, stop=True, skip_group_check=True)

                    # TTS = [tt0s | tt1s] in one tile; E halves -> p0:64,
                    # O halves -> p64:128, each as one (64,512) copy
                    tts = ttp.tile([128, 512], F32R, tag="tts")
                    ttfv = ttf[0:64].rearrange("p (b eo n) -> p b eo n", b=2, eo=2)
                    ttsv = tts.rearrange("p (b n) -> p b n", b=2)
                    nc.scalar.copy(ttsv[0:64], ttfv[:, :, 0])
                    nc.vector.tensor_copy(ttsv[64:128], ttfv[:, :, 1])
                    tt0s = tts[:, 0:256]
                    tt1s = tts[:, 256:512]

                    ypb = ppy.tile([128, 512], F32, tag="ypb")
                    yp0 = ypb[:, 0:256]
                    yp1 = ypb[:, 256:512]
                    nc.tensor.matmul(yp0[:], tt0s[:, 0:128], R_lo,
                                     start=True, stop=False, skip_group_check=True)
                    nc.tensor.matmul(yp0[:], tt1s[:, 0:128], R_hi,
                                     start=False, stop=False, skip_group_check=True)
                    nc.tensor.matmul(yp1[:], tt0s[:, 128:256], R_lo,
                                     start=False, stop=False, skip_group_check=True)
                    nc.tensor.matmul(yp1[:], tt1s[:, 128:256], R_hi,
                                     start=False, stop=True, skip_group_check=True)

                    ocs = slice(ci * 512, (ci + 1) * 512)
                    nc.scalar.copy(YB[:, ocs], ypb[:])

                    if ci % 2 == 1:
                        c0 = g0 + ci - 1
                        fs = (ci - 1) * 512
                        # OUT[c, 0:128] <- YB slice [0:256]; OUT[c,128:256] <- [256:512]
                        nc.gpsimd.dma_start(
                            OUT[c0:c0 + 2].rearrange("g (h p) c -> p g h c", h=2),
                            YB[:, fs:fs + 1024].rearrange(
                                "p (g h c) -> p g h c", g=2, h=2),
                        )

            # software pipeline: emit loads+c2q of group g+1 before the
            # matmul/copy stream of group g
            groups = list(range(0, n_ch, G))
            state = prep_group(load_group(groups[0]))
            next_raw = [None]
            for idx, g0 in enumerate(groups):
                prepped = [None]
                if idx + 1 < len(groups):
                    next_raw[0] = load_group(groups[idx + 1])

                    def mid_emit(nr=next_raw, pr=prepped):
                        pr[0] = prep_group(nr[0])
                    process_group(g0, state, mid_emit)
                    state = prepped[0]
                else:
                    process_group(g0, state)

    nc.compile()
    return nc




# ---------------- host wrapper: shard, run on 8 cores, gather ----------------

_CACHED = {}


def _get_compiled():
    if "nc" not in _CACHED:
        _CACHED["nc"] = build_kernel(n_ch=64, G=8, n_cores=8)
        _CACHED["stats"] = build_statics()
    return _CACHED["nc"], _CACHED["stats"]


def _make_yh(Yhr, Yhi):
    """[C,6,64,64] x2 (fp32) -> [C,6,128,64] with real/imag row-interleave."""
    st = np.stack([Yhr, Yhi], axis=-2)          # [C,6,64,2,64]
    return np.ascontiguousarray(st.reshape(st.shape[0], 6, 128, 64))


def kernel(Yl, Yhr, Yhi):
    """Inverse DTCWT (qshift) level. Yl (8,64,128,128) f32,
    Yhr/Yhi (8,64,6,64,64) f32 -> (8,64,256,256) f32.
    Data-parallel over the batch dim: one batch element per NeuronCore."""
    from concourse.bass_utils import run_bass_kernel_spmd

    Yl = np.ascontiguousarray(np.asarray(Yl, dtype=np.float32))
    Yhr = np.asarray(Yhr, dtype=np.float32)
    Yhi = np.asarray(Yhi, dtype=np.float32)
    B = Yl.shape[0]
    assert B == 8, f"expected batch 8, got {B}"

    nc, (STAT1, STAT2, SIGNS) = _get_compiled()
    in_maps = []
    for b in range(B):
        in_maps.append({
            "Yl": np.ascontiguousarray(Yl[b]),
            "YH": _make_yh(Yhr[b], Yhi[b]),
            "STAT1": STAT1,
            "STAT2": STAT2,
            "SIGNS": SIGNS,
        })
    res = run_bass_kernel_spmd(nc, in_maps, core_ids=list(range(B)))
    out = np.stack([res.results[b]["Y"] for b in range(B)])
    return out.astype(np.float32)



# revision 10
# speedup vs baseline: 1.0835x; 1.0835x over previous
"""DTCWT inverse (qshift, single level) as a Bass/Tile kernel for TRN2.

Per-core computation, per (channel) slice:  Y = Ccat @ Xcat @ Rcat
with Xcat = [[Yl, hl], [lh, hh]] (c2q quadrants), Ccat/Rcat static banded
synthesis matrices, realized as two matmul stages (data stationary, static
matrices moving).

This revision (vs the fp32r baseline):
  - bf16 operands end-to-end (inputs, statics, intermediates, output
    staging); PSUM accumulation stays f32.  Host casts the output back.
  - host-side repack of all inputs into partition-major layouts so every
    DMA line is >=2KB contiguous per partition (baseline averaged 302B
    per DMA packet).
  - stage-1 stationary operands pack a PAIR of channel slices (M=128,
    two 64-col data blocks that share one moving static), halving the
    moving-row count of stage 1 (2048 -> 1024 rows/slice).
  - c2q prep (scalar_tensor_tensor) moved to GpSimd; psum->sbuf copies
    split between Scalar and Vector; output staged bf16 and DMA'd per 4
    slices with 1KB contiguous lines.
"""
import numpy as np
import ml_dtypes

import concourse.bacc as bacc
import concourse.tile as tile
from concourse import mybir

F32 = mybir.dt.float32
BF16 = mybir.dt.bfloat16
BF = ml_dtypes.bfloat16

# ---------------- host-side static matrix construction ----------------

_H0A = np.array([0.0351638365171441, 0.0, -0.0883294244510729,
                 0.233890320607236, 0.760272369066126, 0.587518297723561,
                 0.0, -0.114301837144249, 0.0, 0.0], dtype=np.float64)
_H0B = _H0A[::-1].copy()
_ALT = (-1.0) ** np.arange(10)
_H1A = _H0B * _ALT
_H1B = _H1A[::-1].copy()
G0A, G0B, G1A, G1B = _H0B, _H0A, _H1B, _H1A

PI = np.concatenate([np.arange(0, 128, 2), np.arange(1, 128, 2)])
BANDS = [2, 3, 0, 5, 1, 4]  # (hl.b1, hl.b2, lh.b1, lh.b2, hh.b1, hh.b2)


def _reflect(x, minx, maxx):
    x = np.asarray(x, dtype=np.float64)
    rng = maxx - minx
    rng2 = 2.0 * rng
    mod = np.fmod(x - minx, rng2)
    normed = np.where(mod < 0, mod + rng2, mod)
    return (np.where(normed >= rng, rng2 - normed, normed) + minx).astype(np.int64)


def _colifilt_matrix(ha, hb, r=128):
    """C (2r x r) with colifilt(X) = C @ X."""
    m = ha.shape[0]
    m2 = m // 2
    xe = _reflect(np.arange(-m2, r + m2), -0.5, r - 0.5)
    t = np.arange(2, r + m - 1, 2)
    if float(np.sum(ha * hb)) > 0:
        ta, tb = t, t - 1
    else:
        ta, tb = t - 1, t
    r2 = r // 2
    hao, hae = ha[0::2], ha[1::2]
    hbo, hbe = hb[0::2], hb[1::2]

    def vconv_mat(sel_idx, h):
        hf = h[::-1]
        M = np.zeros((r2, r), dtype=np.float64)
        for i in range(r2):
            for k in range(m2):
                M[i, sel_idx[i + k]] += hf[k]
        return M

    C = np.zeros((2 * r, r), dtype=np.float64)
    C[0::4] = vconv_mat(xe[tb], hao)
    C[1::4] = vconv_mat(xe[ta], hbo)
    C[2::4] = vconv_mat(xe[tb], hae)
    C[3::4] = vconv_mat(xe[ta], hbe)
    return C


def build_statics():
    """STAT1 (128 x 1280) = [S_TL | S_C0_E | S_C0_O | S_C1_E | S_C1_O]
    STAT2 (128 x 512) = [R_lo | R_hi], rows pi-permuted.
    SIGNS (128 x 2): col0 = c2q E-combination sign, col1 = O sign."""
    C0 = _colifilt_matrix(G0B, G0A)
    C1 = _colifilt_matrix(G1B, G1A)
    s = 1.0 / np.sqrt(2.0)
    swap = np.arange(128) ^ 1
    STAT1 = np.concatenate([C0.T, (s * C0).T, (s * C0[:, swap]).T,
                            (s * C1).T, (s * C1[:, swap]).T],
                           axis=1).astype(np.float32)
    STAT2 = np.concatenate([C0.T[PI], C1.T[PI]], axis=1).astype(np.float32)
    SIGNS = np.zeros((128, 2), dtype=np.float32)
    SIGNS[0::2, 0] = 1.0
    SIGNS[1::2, 0] = -1.0
    SIGNS[0::2, 1] = -1.0
    SIGNS[1::2, 1] = 1.0
    return (np.ascontiguousarray(STAT1.astype(BF)),
            np.ascontiguousarray(STAT2.astype(BF)),
            np.ascontiguousarray(SIGNS.astype(BF)))


# ---------------- device kernel ----------------


def build_kernel(n_ch=64, G=8, n_cores=8):
    """Per-core Bass module; processes n_ch channel slices in groups of G."""
    nc = bacc.Bacc("TRN2", target_bir_lowering=False, debug=False,
                   num_devices=n_cores)
    YL2 = nc.dram_tensor("YL2", [128, n_ch // 2, 2, 128], BF16,
                         kind="ExternalInput").ap()
    YH2 = nc.dram_tensor("YH2", [128, n_ch, 6, 64], BF16,
                         kind="ExternalInput").ap()
    ST1 = nc.dram_tensor("STAT1", [128, 1280], BF16, kind="ExternalInput").ap()
    ST2 = nc.dram_tensor("STAT2", [128, 512], BF16, kind="ExternalInput").ap()
    SGN = nc.dram_tensor("SIGNS", [128, 2], BF16, kind="ExternalInput").ap()
    OUT = nc.dram_tensor("Y", [n_ch, 128, 512], BF16,
                         kind="ExternalOutput").ap()

    assert n_ch % G == 0 and G % 2 == 0
    n_groups = n_ch // G
    pairs_per_group = G // 2
    n_pairs = n_ch // 2
    mm = None

    with tile.TileContext(nc) as tc:
        with (
            tc.tile_pool(name="const", bufs=1) as const,
            tc.tile_pool(name="inp", bufs=2) as inp,
            tc.tile_pool(name="quad", bufs=2) as quad,
            tc.tile_pool(name="tt", bufs=3) as ttp,
            tc.tile_pool(name="yout", bufs=2) as yp,
            tc.tile_pool(name="psum", bufs=2, space="PSUM") as pp,
            tc.tile_pool(name="psumy", bufs=2, space="PSUM") as ppy,
        ):
            s1 = const.tile([128, 1280], BF16)
            nc.sync.dma_start(s1[:], ST1[:])
            s2 = const.tile([128, 512], BF16)
            nc.sync.dma_start(s2[:], ST2[:])
            sgn = const.tile([128, 2], BF16)
            nc.sync.dma_start(sgn[:], SGN[:])

            S_TL = s1[:, 0:256]
            S_E = {"C0": s1[:, 256:512], "C1": s1[:, 768:1024]}
            S_O = {"C0": s1[:, 512:768], "C1": s1[:, 1024:1280]}
            R_lo = s2[:, 0:256]
            R_hi = s2[:, 256:512]
            s_a = sgn[:, 0:1]
            s_b = sgn[:, 1:2]
            mm = nc.tensor.matmul

            def load_group(g0):
                TL = inp.tile([128, 128 * G], BF16, tag="TL")
                nc.sync.dma_start(
                    TL.rearrange("p (m eo j) -> p m eo j", m=G // 2, eo=2),
                    YL2[:, g0 // 2:(g0 + G) // 2])
                YHT = inp.tile([128, 384 * G], BF16, tag="YHT")
                nc.sync.dma_start(
                    YHT.rearrange("p (g k c) -> p g k c", g=G, k=6),
                    YH2[:, g0:g0 + G])
                return TL, YHT

            def prep_group(raw):
                TL, YHT = raw
                YHv = YHT.rearrange("p (g k c) -> p g k c", g=G, k=6)
                D = {}
                for qi, q in enumerate(("hl", "lh", "hh")):
                    DE = quad.tile([128, 64 * G], BF16, tag=f"DE_{q}")
                    DO = quad.tile([128, 64 * G], BF16, tag=f"DO_{q}")
                    nc.vector.scalar_tensor_tensor(
                        DE.rearrange("p (g c) -> p g c", g=G),
                        YHv[:, :, 2 * qi + 1], s_a, YHv[:, :, 2 * qi],
                        op0=mybir.AluOpType.mult, op1=mybir.AluOpType.add)
                    nc.vector.scalar_tensor_tensor(
                        DO.rearrange("p (g c) -> p g c", g=G),
                        YHv[:, :, 2 * qi], s_b, YHv[:, :, 2 * qi + 1],
                        op0=mybir.AluOpType.mult, op1=mybir.AluOpType.add)
                    D[q] = (DE, DO)
                return TL, D

            def emit_stage1(state, kpl):
                """Stage 1 for local pair kpl: 8 matmuls (paired stationary,
                M=128) into a 2-bank psum tile, then 4 psum->sbuf copies."""
                TL, D = state
                c0 = 2 * kpl
                TLv = TL.rearrange("p (m eo j) -> p m eo j", m=G // 2, eo=2)
                qs = slice(c0 * 64, (c0 + 2) * 64)
                hlE, hlO = D["hl"]
                lhE, lhO = D["lh"]
                hhE, hhO = D["hh"]
                P = pp.tile([128, 1024], F32, tag="P")
                # bank 0: tt0 = (C0@Yl + C1@lh)^T, [E cols | O cols]
                mm(P[:, 0:256], TLv[:, kpl, 0], S_TL,
                   start=True, stop=False, skip_group_check=True)
                mm(P[:, 256:512], TLv[:, kpl, 1], S_TL,
                   start=False, stop=False, skip_group_check=True)
                mm(P[:, 0:256], lhE[:, qs], S_E["C1"],
                   start=False, stop=False, skip_group_check=True)
                mm(P[:, 256:512], lhO[:, qs], S_O["C1"],
                   start=False, stop=True, skip_group_check=True)
                # bank 1: tt1 = (C0@hl + C1@hh)^T
                mm(P[:, 512:768], hlE[:, qs], S_E["C0"],
                   start=True, stop=False, skip_group_check=True)
                mm(P[:, 768:1024], hlO[:, qs], S_O["C0"],
                   start=False, stop=False, skip_group_check=True)
                mm(P[:, 512:768], hhE[:, qs], S_E["C1"],
                   start=False, stop=False, skip_group_check=True)
                mm(P[:, 768:1024], hhO[:, qs], S_O["C1"],
                   start=False, stop=True, skip_group_check=True)

                # psum partitions 0:64 = slice c0, 64:128 = slice c0+1.
                # free layout (b, eo, n): b in {tt0, tt1}, eo in {E, O}.
                Pv = P.rearrange("p (b eo n) -> p b eo n", b=2, eo=2)
                t0 = ttp.tile([128, 512], BF16, tag="tts0")
                t1 = ttp.tile([128, 512], BF16, tag="tts1")
                t0v = t0.rearrange("p (b n) -> p b n", b=2)
                t1v = t1.rearrange("p (b n) -> p b n", b=2)
                nc.scalar.copy(t0v[0:64], Pv[0:64, :, 0])
                nc.vector.tensor_copy(t0v[64:128], Pv[0:64, :, 1])
                nc.vector.tensor_copy(t1v[0:64], Pv[64:128, :, 0])
                nc.scalar.copy(t1v[64:128], Pv[64:128, :, 1])
                return t0, t1

            yb_state = {}

            def emit_stage2(pend, kp):
                """Stage 2 for global pair kp (channels 2kp, 2kp+1): 8
                matmuls, bf16 staging copy, and per-4-slice output DMA."""
                t0, t1 = pend
                YPP = ppy.tile([128, 1024], F32, tag="YPP")
                for ci, t in ((0, t0), (1, t1)):
                    b = ci * 512
                    mm(YPP[:, b:b + 256], t[:, 0:128], R_lo,
                       start=True, stop=False, skip_group_check=True)
                    mm(YPP[:, b:b + 256], t[:, 256:384], R_hi,
                       start=False, stop=False, skip_group_check=True)
                    mm(YPP[:, b + 256:b + 512], t[:, 128:256], R_lo,
                       start=False, stop=False, skip_group_check=True)
                    mm(YPP[:, b + 256:b + 512], t[:, 384:512], R_hi,
                       start=False, stop=True, skip_group_check=True)
                if kp % 2 == 0:
                    yb_new = yp.tile([128, 2048], BF16, tag="YB")
                    yb_state["tile"] = yb_new
                YB = yb_state["tile"]
                half = (kp % 2) * 1024
                nc.scalar.copy(YB[:, half:half + 1024], YPP[:])
                if kp % 2 == 1:
                    c4 = (kp - 1) * 2
                    nc.gpsimd.dma_start(
                        OUT[c4:c4 + 4].rearrange("g p f -> p g f"),
                        YB.rearrange("p (g f) -> p g f", g=4))

            # software pipeline: stage 2 trails stage 1 by one pair; next
            # group's loads at pair 0, its c2q prep at pair 2.
            state = prep_group(load_group(0))
            nxt_raw = None
            nxt_state = None
            pend = None
            for kp in range(n_pairs):
                g, kpl = divmod(kp, pairs_per_group)
                if kpl == 0 and g + 1 < n_groups:
                    nxt_raw = load_group((g + 1) * G)
                cur = emit_stage1(state, kpl)
                if pend is not None:
                    emit_stage2(pend, kp - 1)
                pend = cur
                if kpl == 2 and g + 1 < n_groups:
                    nxt_state = prep_group(nxt_raw)
                if kpl == pairs_per_group - 1 and g + 1 < n_groups:
                    state = nxt_state
            emit_stage2(pend, n_pairs - 1)

    nc.compile()
    return nc


# ---------------- host wrapper: shard, run on 8 cores, gather ----------------

_CACHED = {}


def _get_compiled():
    if "nc" not in _CACHED:
        _CACHED["nc"] = build_kernel(n_ch=64, G=8, n_cores=8)
        _CACHED["stats"] = build_statics()
    return _CACHED["nc"], _CACHED["stats"]


def _pack_inputs(Yl_b, Yhr_b, Yhi_b):
    """Per-batch host repack into partition-major bf16 layouts.
    Yl: [p, pair, {E|O}, 128] with each 128-block = [cols(2m) | cols(2m+1)]
    so a channel-pair stationary is one contiguous slice."""
    t = Yl_b.transpose(1, 0, 2)                      # [p, ch, c]
    e = t[..., 0::2].reshape(128, 32, 128)
    o = t[..., 1::2].reshape(128, 32, 128)
    yl2 = np.stack([e, o], axis=2)                   # [p, 32, 2, 128]
    st = np.stack([Yhr_b[:, BANDS], Yhi_b[:, BANDS]], axis=3)  # [ch,6,64,2,64]
    yh2 = st.transpose(2, 3, 0, 1, 4).reshape(128, Yl_b.shape[0], 6, 64)
    return (np.ascontiguousarray(yl2.astype(BF)),
            np.ascontiguousarray(yh2.astype(BF)))


def _make_in_maps(Yl, Yhr, Yhi, stats):
    STAT1, STAT2, SIGNS = stats
    in_maps = []
    for b in range(Yl.shape[0]):
        yl2, yh2 = _pack_inputs(Yl[b], Yhr[b], Yhi[b])
        in_maps.append({"YL2": yl2, "YH2": yh2, "STAT1": STAT1,
                        "STAT2": STAT2, "SIGNS": SIGNS})
    return in_maps


def _unpack_out(res_y):
    # [ch, p(128), (h c)] bf16 -> [ch, 256, 256] f32, row = h*128 + p
    y = np.asarray(res_y).astype(np.float32).reshape(64, 128, 2, 256)
    return y.transpose(0, 2, 1, 3).reshape(64, 256, 256)


def kernel(Yl, Yhr, Yhi):
    """Inverse DTCWT (qshift) level. Yl (8,64,128,128) f32,
    Yhr/Yhi (8,64,6,64,64) f32 -> (8,64,256,256) f32.
    Data-parallel over the batch dim: one batch element per NeuronCore."""
    from concourse.bass_utils import run_bass_kernel_spmd

    Yl = np.asarray(Yl, dtype=np.float32)
    Yhr = np.asarray(Yhr, dtype=np.float32)
    Yhi = np.asarray(Yhi, dtype=np.float32)
    B = Yl.shape[0]
    assert B == 8, f"expected batch 8, got {B}"

    nc, stats = _get_compiled()
    in_maps = _make_in_maps(Yl, Yhr, Yhi, stats)
    res = run_bass_kernel_spmd(nc, in_maps, core_ids=list(range(B)))
    out = np.stack([_unpack_out(res.results[b]["Y"]) for b in range(B)])
    return out.astype(np.float32)


# revision 15
# speedup vs baseline: 1.0977x; 1.0131x over previous
"""DTCWT inverse (qshift, single level) as a Bass/Tile kernel for TRN2.

Per-core computation, per (channel) slice:  Y = Ccat @ Xcat @ Rcat
with Xcat = [[Yl, hl], [lh, hh]] (c2q quadrants), Ccat/Rcat static banded
synthesis matrices, realized as two matmul stages (data stationary, static
matrices moving).

This revision (vs the fp32r baseline):
  - bf16 operands end-to-end (inputs, statics, intermediates, output
    staging); PSUM accumulation stays f32.  Host casts the output back.
  - host-side repack of all inputs into partition-major layouts so every
    DMA line is >=2KB contiguous per partition (baseline averaged 302B
    per DMA packet).
  - stage-1 stationary operands pack a PAIR of channel slices (M=128,
    two 64-col data blocks that share one moving static), halving the
    moving-row count of stage 1 (2048 -> 1024 rows/slice).
  - c2q prep (scalar_tensor_tensor) moved to GpSimd; psum->sbuf copies
    split between Scalar and Vector; output staged bf16 and DMA'd per 4
    slices with 1KB contiguous lines.
"""
import numpy as np
import ml_dtypes

import concourse.bacc as bacc
import concourse.tile as tile
from concourse import mybir

F32 = mybir.dt.float32
BF16 = mybir.dt.bfloat16
BF = ml_dtypes.bfloat16

# ---------------- host-side static matrix construction ----------------

_H0A = np.array([0.0351638365171441, 0.0, -0.0883294244510729,
                 0.233890320607236, 0.760272369066126, 0.587518297723561,
                 0.0, -0.114301837144249, 0.0, 0.0], dtype=np.float64)
_H0B = _H0A[::-1].copy()
_ALT = (-1.0) ** np.arange(10)
_H1A = _H0B * _ALT
_H1B = _H1A[::-1].copy()
G0A, G0B, G1A, G1B = _H0B, _H0A, _H1B, _H1A

PI = np.concatenate([np.arange(0, 128, 2), np.arange(1, 128, 2)])
BANDS = [2, 3, 0, 5, 1, 4]  # (hl.b1, hl.b2, lh.b1, lh.b2, hh.b1, hh.b2)


def _reflect(x, minx, maxx):
    x = np.asarray(x, dtype=np.float64)
    rng = maxx - minx
    rng2 = 2.0 * rng
    mod = np.fmod(x - minx, rng2)
    normed = np.where(mod < 0, mod + rng2, mod)
    return (np.where(normed >= rng, rng2 - normed, normed) + minx).astype(np.int64)


def _colifilt_matrix(ha, hb, r=128):
    """C (2r x r) with colifilt(X) = C @ X."""
    m = ha.shape[0]
    m2 = m // 2
    xe = _reflect(np.arange(-m2, r + m2), -0.5, r - 0.5)
    t = np.arange(2, r + m - 1, 2)
    if float(np.sum(ha * hb)) > 0:
        ta, tb = t, t - 1
    else:
        ta, tb = t - 1, t
    r2 = r // 2
    hao, hae = ha[0::2], ha[1::2]
    hbo, hbe = hb[0::2], hb[1::2]

    def vconv_mat(sel_idx, h):
        hf = h[::-1]
        M = np.zeros((r2, r), dtype=np.float64)
        for i in range(r2):
            for k in range(m2):
                M[i, sel_idx[i + k]] += hf[k]
        return M

    C = np.zeros((2 * r, r), dtype=np.float64)
    C[0::4] = vconv_mat(xe[tb], hao)
    C[1::4] = vconv_mat(xe[ta], hbo)
    C[2::4] = vconv_mat(xe[tb], hae)
    C[3::4] = vconv_mat(xe[ta], hbe)
    return C


def build_statics():
    """STAT1 (128 x 1280) = [S_TL | S_C0_E | S_C0_O | S_C1_E | S_C1_O]
    STAT2 (128 x 512) = [R_lo | R_hi], rows pi-permuted.
    SIGNS (128 x 2): col0 = c2q E-combination sign, col1 = O sign."""
    C0 = _colifilt_matrix(G0B, G0A)
    C1 = _colifilt_matrix(G1B, G1A)
    s = 1.0 / np.sqrt(2.0)
    swap = np.arange(128) ^ 1
    STAT1 = np.concatenate([C0.T, (s * C0).T, (s * C0[:, swap]).T,
                            (s * C1).T, (s * C1[:, swap]).T],
                           axis=1).astype(np.float32)
    STAT2 = np.concatenate([C0.T[PI], C1.T[PI]], axis=1).astype(np.float32)
    SIGNS = np.zeros((128, 2), dtype=np.float32)
    SIGNS[0::2, 0] = 1.0
    SIGNS[1::2, 0] = -1.0
    SIGNS[0::2, 1] = -1.0
    SIGNS[1::2, 1] = 1.0
    return (np.ascontiguousarray(STAT1.astype(BF)),
            np.ascontiguousarray(STAT2.astype(BF)),
            np.ascontiguousarray(SIGNS.astype(BF)))


# ---------------- device kernel ----------------


def build_kernel(n_ch=64, G=8, n_cores=8):
    """Per-core Bass module; processes n_ch channel slices in groups of G."""
    nc = bacc.Bacc("TRN2", target_bir_lowering=False, debug=False,
                   num_devices=n_cores)
    YL2 = nc.dram_tensor("YL2", [128, n_ch // 2, 2, 128], BF16,
                         kind="ExternalInput").ap()
    YH2 = nc.dram_tensor("YH2", [128, 6, n_ch, 64], BF16,
                         kind="ExternalInput").ap()
    ST1 = nc.dram_tensor("STAT1", [128, 1280], BF16, kind="ExternalInput").ap()
    ST2 = nc.dram_tensor("STAT2", [128, 512], BF16, kind="ExternalInput").ap()
    SGN = nc.dram_tensor("SIGNS", [128, 2], BF16, kind="ExternalInput").ap()
    OUT = nc.dram_tensor("Y", [n_ch, 128, 512], BF16,
                         kind="ExternalOutput").ap()

    assert n_ch % G == 0 and G % 2 == 0
    n_groups = n_ch // G
    pairs_per_group = G // 2
    n_pairs = n_ch // 2
    mm = None

    with tile.TileContext(nc) as tc:
        with (
            tc.tile_pool(name="const", bufs=1) as const,
            tc.tile_pool(name="inp", bufs=2) as inp,
            tc.tile_pool(name="quad", bufs=2) as quad,
            tc.tile_pool(name="tt", bufs=4) as ttp,
            tc.tile_pool(name="yout", bufs=2) as yp,
            tc.tile_pool(name="psum", bufs=2, space="PSUM") as pp,
            tc.tile_pool(name="psumy", bufs=2, space="PSUM") as ppy,
        ):
            s1 = const.tile([128, 1280], BF16)
            nc.sync.dma_start(s1[:], ST1[:])
            s2 = const.tile([128, 512], BF16)
            nc.sync.dma_start(s2[:], ST2[:])
            sgn = const.tile([128, 2], BF16)
            nc.sync.dma_start(sgn[:], SGN[:])

            S_TL = s1[:, 0:256]
            S_E = {"C0": s1[:, 256:512], "C1": s1[:, 768:1024]}
            S_O = {"C0": s1[:, 512:768], "C1": s1[:, 1024:1280]}
            R_lo = s2[:, 0:256]
            R_hi = s2[:, 256:512]
            s_a = sgn[:, 0:1]
            s_b = sgn[:, 1:2]
            mm = nc.tensor.matmul

            def load_group(g0):
                TL = inp.tile([128, 128 * G], BF16, tag="TL")
                nc.sync.dma_start(
                    TL.rearrange("p (m eo j) -> p m eo j", m=G // 2, eo=2),
                    YL2[:, g0 // 2:(g0 + G) // 2])
                YHT = inp.tile([128, 384 * G], BF16, tag="YHT")
                nc.sync.dma_start(
                    YHT.rearrange("p (k g c) -> p k g c", k=6, g=G),
                    YH2[:, :, g0:g0 + G])
                return TL, YHT

            def prep_quad(raw, D, qi, q):
                """c2q for one quadrant of a group: 2 contiguous STTs."""
                _TL, YHT = raw
                YHv = YHT.rearrange("p (k gc) -> p k gc", k=6)
                DE = quad.tile([128, 64 * G], BF16, tag=f"DE_{q}")
                DO = quad.tile([128, 64 * G], BF16, tag=f"DO_{q}")
                nc.vector.scalar_tensor_tensor(
                    DE[:], YHv[:, 2 * qi + 1], s_a, YHv[:, 2 * qi],
                    op0=mybir.AluOpType.mult, op1=mybir.AluOpType.add)
                nc.vector.scalar_tensor_tensor(
                    DO[:], YHv[:, 2 * qi], s_b, YHv[:, 2 * qi + 1],
                    op0=mybir.AluOpType.mult, op1=mybir.AluOpType.add)
                D[q] = (DE, DO)

            def prep_group(raw):
                D = {}
                for qi, q in enumerate(("hl", "lh", "hh")):
                    prep_quad(raw, D, qi, q)
                return raw[0], D

            def emit_stage1(state, kpl):
                """Stage 1 for local pair kpl: 8 matmuls (paired stationary,
                M=128) into a 2-bank psum tile, then 4 psum->sbuf copies."""
                TL, D = state
                c0 = 2 * kpl
                TLv = TL.rearrange("p (m eo j) -> p m eo j", m=G // 2, eo=2)
                qs = slice(c0 * 64, (c0 + 2) * 64)
                hlE, hlO = D["hl"]
                lhE, lhO = D["lh"]
                hhE, hhO = D["hh"]
                P = pp.tile([128, 1024], F32, tag="P")
                # bank 0: tt0 = (C0@Yl + C1@lh)^T, [E cols | O cols]
                mm(P[:, 0:256], TLv[:, kpl, 0], S_TL,
                   start=True, stop=False, skip_group_check=True)
                mm(P[:, 256:512], TLv[:, kpl, 1], S_TL,
                   start=False, stop=False, skip_group_check=True)
                mm(P[:, 0:256], lhE[:, qs], S_E["C1"],
                   start=False, stop=False, skip_group_check=True)
                mm(P[:, 256:512], lhO[:, qs], S_O["C1"],
                   start=False, stop=True, skip_group_check=True)
                # bank 1: tt1 = (C0@hl + C1@hh)^T
                mm(P[:, 512:768], hlE[:, qs], S_E["C0"],
                   start=True, stop=False, skip_group_check=True)
                mm(P[:, 768:1024], hlO[:, qs], S_O["C0"],
                   start=False, stop=False, skip_group_check=True)
                mm(P[:, 512:768], hhE[:, qs], S_E["C1"],
                   start=False, stop=False, skip_group_check=True)
                mm(P[:, 768:1024], hhO[:, qs], S_O["C1"],
                   start=False, stop=True, skip_group_check=True)

                # psum partitions 0:64 = slice c0, 64:128 = slice c0+1.
                # free layout (b, eo, n): b in {tt0, tt1}, eo in {E, O}.
                Pv = P.rearrange("p (b eo n) -> p b eo n", b=2, eo=2)
                t0 = ttp.tile([128, 512], BF16, tag="tts0")
                t1 = ttp.tile([128, 512], BF16, tag="tts1")
                t0v = t0.rearrange("p (b n) -> p b n", b=2)
                t1v = t1.rearrange("p (b n) -> p b n", b=2)
                nc.scalar.copy(t0v[0:64], Pv[0:64, :, 0])
                nc.vector.tensor_copy(t0v[64:128], Pv[0:64, :, 1])
                nc.vector.tensor_copy(t1v[0:64], Pv[64:128, :, 0])
                nc.scalar.copy(t1v[64:128], Pv[64:128, :, 1])
                return t0, t1

            yb_state = {}

            def emit_stage2(pend, kp):
                """Stage 2 for global pair kp (channels 2kp, 2kp+1): 8
                matmuls, bf16 staging copy, and per-4-slice output DMA."""
                t0, t1 = pend
                YPP = ppy.tile([128, 1024], F32, tag="YPP")
                for ci, t in ((0, t0), (1, t1)):
                    b = ci * 512
                    mm(YPP[:, b:b + 256], t[:, 0:128], R_lo,
                       start=True, stop=False, skip_group_check=True)
                    mm(YPP[:, b:b + 256], t[:, 256:384], R_hi,
                       start=False, stop=False, skip_group_check=True)
                    mm(YPP[:, b + 256:b + 512], t[:, 128:256], R_lo,
                       start=False, stop=False, skip_group_check=True)
                    mm(YPP[:, b + 256:b + 512], t[:, 384:512], R_hi,
                       start=False, stop=True, skip_group_check=True)
                if kp % 2 == 0:
                    yb_new = yp.tile([128, 2048], BF16, tag="YB")
                    yb_state["tile"] = yb_new
                YB = yb_state["tile"]
                half = (kp % 2) * 1024
                nc.scalar.copy(YB[:, half:half + 1024], YPP[:])
                if kp % 2 == 1:
                    c4 = (kp - 1) * 2
                    nc.gpsimd.dma_start(
                        OUT[c4:c4 + 4].rearrange("g p f -> p g f"),
                        YB.rearrange("p (g f) -> p g f", g=4))

            # software pipeline: stage 2 trails stage 1 by one pair; next
            # group's loads at pair 0, its c2q prep at pair 2.
            state = prep_group(load_group(0))
            nxt_raw = None
            nxt_D = None
            pend = None
            for kp in range(n_pairs):
                g, kpl = divmod(kp, pairs_per_group)
                if kpl == 0 and g + 1 < n_groups:
                    nxt_raw = load_group((g + 1) * G)
                    nxt_D = {}
                cur = emit_stage1(state, kpl)
                if pend is not None:
                    emit_stage2(pend, kp - 1)
                pend = cur
                if g + 1 < n_groups and 1 <= kpl <= 3:
                    qi = kpl - 1
                    prep_quad(nxt_raw, nxt_D, qi, ("hl", "lh", "hh")[qi])
                if kpl == pairs_per_group - 1 and g + 1 < n_groups:
                    state = (nxt_raw[0], nxt_D)
            emit_stage2(pend, n_pairs - 1)

    nc.compile()
    return nc


# ---------------- host wrapper: shard, run on 8 cores, gather ----------------

_CACHED = {}


def _get_compiled():
    if "nc" not in _CACHED:
        _CACHED["nc"] = build_kernel(n_ch=64, G=8, n_cores=8)
        _CACHED["stats"] = build_statics()
    return _CACHED["nc"], _CACHED["stats"]


def _pack_inputs(Yl_b, Yhr_b, Yhi_b):
    """Per-batch host repack into partition-major bf16 layouts.
    Yl: [p, pair, {E|O}, 128] with each 128-block = [cols(2m) | cols(2m+1)]
    so a channel-pair stationary is one contiguous slice."""
    t = Yl_b.transpose(1, 0, 2)                      # [p, ch, c]
    e = t[..., 0::2].reshape(128, 32, 128)
    o = t[..., 1::2].reshape(128, 32, 128)
    yl2 = np.stack([e, o], axis=2)                   # [p, 32, 2, 128]
    st = np.stack([Yhr_b[:, BANDS], Yhi_b[:, BANDS]], axis=3)  # [ch,6,64,2,64]
    yh2 = st.transpose(2, 3, 1, 0, 4).reshape(128, 6, Yl_b.shape[0], 64)
    return (np.ascontiguousarray(yl2.astype(BF)),
            np.ascontiguousarray(yh2.astype(BF)))


def _make_in_maps(Yl, Yhr, Yhi, stats):
    STAT1, STAT2, SIGNS = stats
    in_maps = []
    for b in range(Yl.shape[0]):
        yl2, yh2 = _pack_inputs(Yl[b], Yhr[b], Yhi[b])
        in_maps.append({"YL2": yl2, "YH2": yh2, "STAT1": STAT1,
                        "STAT2": STAT2, "SIGNS": SIGNS})
    return in_maps


def _unpack_out(res_y):
    # [ch, p(128), (h c)] bf16 -> [ch, 256, 256] f32, row = h*128 + p
    y = np.asarray(res_y).astype(np.float32).reshape(64, 128, 2, 256)
    return y.transpose(0, 2, 1, 3).reshape(64, 256, 256)


def kernel(Yl, Yhr, Yhi):
    """Inverse DTCWT (qshift) level. Yl (8,64,128,128) f32,
    Yhr/Yhi (8,64,6,64,64) f32 -> (8,64,256,256) f32.
    Data-parallel over the batch dim: one batch element per NeuronCore."""
    from concourse.bass_utils import run_bass_kernel_spmd

    Yl = np.asarray(Yl, dtype=np.float32)
    Yhr = np.asarray(Yhr, dtype=np.float32)
    Yhi = np.asarray(Yhi, dtype=np.float32)
    B = Yl.shape[0]
    assert B == 8, f"expected batch 8, got {B}"

    nc, stats = _get_compiled()
    in_maps = _make_in_maps(Yl, Yhr, Yhi, stats)
    res = run_bass_kernel_spmd(nc, in_maps, core_ids=list(range(B)))
    out = np.stack([_unpack_out(res.results[b]["Y"]) for b in range(B)])
    return out.astype(np.float32)


# revision 16
# speedup vs baseline: 1.2344x; 1.1246x over previous
"""DTCWT inverse (qshift, single level) as a Bass/Tile kernel for TRN2.

Per-core computation, per channel slice:  Y = Ccat @ Xcat @ Rcat
with Xcat = [[Yl, hl], [lh, hh]] (c2q quadrants), Ccat/Rcat static banded
synthesis matrices, realized as two matmul stages (data stationary, static
matrices moving).

Key structural choices:
  - bf16 operands end-to-end (PSUM accumulation stays f32); host casts the
    bf16 output back to f32.
  - host repacks all inputs into partition-major layouts (>=1KB contiguous
    DMA lines) and stores YH twice: once re/im-interleaved on partitions
    and once im/re-interleaved.  With the swapped copy, the odd-column c2q
    rows come out UNswapped, so even and odd quad columns share the same
    moving static, and the per-slice stationary is one contiguous
    [E-cols | O-cols] block of 128.
  - stage 1 is then 4 matmuls per slice (M=128, N=256) into one PSUM bank
    whose layout equals what stage 2 needs: a single full-width [128,512]
    psum->sbuf copy per slice (no partition remaps).
  - stage 2: 4 matmuls per slice; output staged bf16 and DMA'd per 4
    slices with 1KB contiguous lines.
"""
import numpy as np
import ml_dtypes

import concourse.bacc as bacc
import concourse.tile as tile
from concourse import mybir

F32 = mybir.dt.float32
BF16 = mybir.dt.bfloat16
BF = ml_dtypes.bfloat16

# ---------------- host-side static matrix construction ----------------

_H0A = np.array([0.0351638365171441, 0.0, -0.0883294244510729,
                 0.233890320607236, 0.760272369066126, 0.587518297723561,
                 0.0, -0.114301837144249, 0.0, 0.0], dtype=np.float64)
_H0B = _H0A[::-1].copy()
_ALT = (-1.0) ** np.arange(10)
_H1A = _H0B * _ALT
_H1B = _H1A[::-1].copy()
G0A, G0B, G1A, G1B = _H0B, _H0A, _H1B, _H1A

PI = np.concatenate([np.arange(0, 128, 2), np.arange(1, 128, 2)])
BANDS = [2, 3, 0, 5, 1, 4]  # (hl.b1, hl.b2, lh.b1, lh.b2, hh.b1, hh.b2)


def _reflect(x, minx, maxx):
    x = np.asarray(x, dtype=np.float64)
    rng = maxx - minx
    rng2 = 2.0 * rng
    mod = np.fmod(x - minx, rng2)
    normed = np.where(mod < 0, mod + rng2, mod)
    return (np.where(normed >= rng, rng2 - normed, normed) + minx).astype(np.int64)


def _colifilt_matrix(ha, hb, r=128):
    """C (2r x r) with colifilt(X) = C @ X."""
    m = ha.shape[0]
    m2 = m // 2
    xe = _reflect(np.arange(-m2, r + m2), -0.5, r - 0.5)
    t = np.arange(2, r + m - 1, 2)
    if float(np.sum(ha * hb)) > 0:
        ta, tb = t, t - 1
    else:
        ta, tb = t - 1, t
    r2 = r // 2
    hao, hae = ha[0::2], ha[1::2]
    hbo, hbe = hb[0::2], hb[1::2]

    def vconv_mat(sel_idx, h):
        hf = h[::-1]
        M = np.zeros((r2, r), dtype=np.float64)
        for i in range(r2):
            for k in range(m2):
                M[i, sel_idx[i + k]] += hf[k]
        return M

    C = np.zeros((2 * r, r), dtype=np.float64)
    C[0::4] = vconv_mat(xe[tb], hao)
    C[1::4] = vconv_mat(xe[ta], hbo)
    C[2::4] = vconv_mat(xe[tb], hae)
    C[3::4] = vconv_mat(xe[ta], hbe)
    return C


def build_statics():
    """STAT1 (128 x 768) = [S_TL | S_E0 | S_E1]; STAT2 (128 x 512) =
    [R_lo | R_hi] rows pi-permuted; SIGNS (128 x 1) c2q signs."""
    C0 = _colifilt_matrix(G0B, G0A)
    C1 = _colifilt_matrix(G1B, G1A)
    s = 1.0 / np.sqrt(2.0)
    STAT1 = np.concatenate([C0.T, (s * C0).T, (s * C1).T],
                           axis=1).astype(np.float32)
    STAT2 = np.concatenate([C0.T[PI], C1.T[PI]], axis=1).astype(np.float32)
    SIGNS = np.where(np.arange(128) % 2 == 0, 1.0, -1.0)[:, None]
    return (np.ascontiguousarray(STAT1.astype(BF)),
            np.ascontiguousarray(STAT2.astype(BF)),
            np.ascontiguousarray(SIGNS.astype(BF)))


# ---------------- device kernel ----------------


def build_kernel(n_ch=64, G=8, n_cores=8):
    """Per-core Bass module; processes n_ch channel slices in groups of G."""
    nc = bacc.Bacc("TRN2", target_bir_lowering=False, debug=False,
                   num_devices=n_cores)
    YL2 = nc.dram_tensor("YL2", [128, n_ch, 128], BF16,
                         kind="ExternalInput").ap()
    YHN = nc.dram_tensor("YHN", [128, 6, n_ch, 64], BF16,
                         kind="ExternalInput").ap()
    YHS = nc.dram_tensor("YHS", [128, 6, n_ch, 64], BF16,
                         kind="ExternalInput").ap()
    ST1 = nc.dram_tensor("STAT1", [128, 768], BF16, kind="ExternalInput").ap()
    ST2 = nc.dram_tensor("STAT2", [128, 512], BF16, kind="ExternalInput").ap()
    SGN = nc.dram_tensor("SIGNS", [128, 1], BF16, kind="ExternalInput").ap()
    OUT = nc.dram_tensor("Y", [n_ch, 128, 512], BF16,
                         kind="ExternalOutput").ap()

    assert n_ch % G == 0 and G % 2 == 0
    n_groups = n_ch // G

    with tile.TileContext(nc) as tc:
        with (
            tc.tile_pool(name="const", bufs=1) as const,
            tc.tile_pool(name="inp", bufs=2) as inp,
            tc.tile_pool(name="quad", bufs=2) as quad,
            tc.tile_pool(name="tt", bufs=4) as ttp,
            tc.tile_pool(name="yout", bufs=2) as yp,
            tc.tile_pool(name="psum", bufs=4, space="PSUM") as pp,
            tc.tile_pool(name="psumy", bufs=2, space="PSUM") as ppy,
        ):
            s1 = const.tile([128, 768], BF16)
            nc.sync.dma_start(s1[:], ST1[:])
            s2 = const.tile([128, 512], BF16)
            nc.sync.dma_start(s2[:], ST2[:])
            sgn = const.tile([128, 1], BF16)
            nc.sync.dma_start(sgn[:], SGN[:])

            S_TL = s1[:, 0:256]
            S_E0 = s1[:, 256:512]
            S_E1 = s1[:, 512:768]
            R_lo = s2[:, 0:256]
            R_hi = s2[:, 256:512]
            s_a = sgn[:, 0:1]
            mm = nc.tensor.matmul

            def load_group(g0):
                TL = inp.tile([128, 128 * G], BF16, tag="TL")
                nc.sync.dma_start(
                    TL.rearrange("p (g x) -> p g x", g=G),
                    YL2[:, g0:g0 + G])
                HN = inp.tile([128, 384 * G], BF16, tag="HN")
                nc.sync.dma_start(
                    HN.rearrange("p (k g c) -> p k g c", k=6, g=G),
                    YHN[:, :, g0:g0 + G])
                HS = inp.tile([128, 384 * G], BF16, tag="HS")
                nc.sync.dma_start(
                    HS.rearrange("p (k g c) -> p k g c", k=6, g=G),
                    YHS[:, :, g0:g0 + G])
                return TL, HN, HS

            def prep_quad(raw, D, qi, q):
                """c2q for one quadrant of a group: E from the natural
                interleave, O (unswapped) from the swapped interleave."""
                _TL, HN, HS = raw
                hnv = HN.rearrange("p (k g c) -> p k g c", k=6, g=G)
                hsv = HS.rearrange("p (k g c) -> p k g c", k=6, g=G)
                DQ = quad.tile([128, 128 * G], BF16, tag=f"DQ_{q}")
                dv = DQ.rearrange("p (g eo c) -> p g eo c", g=G, eo=2)
                nc.vector.scalar_tensor_tensor(
                    dv[:, :, 0], hnv[:, 2 * qi + 1], s_a, hnv[:, 2 * qi],
                    op0=mybir.AluOpType.mult, op1=mybir.AluOpType.add)
                nc.vector.scalar_tensor_tensor(
                    dv[:, :, 1], hsv[:, 2 * qi], s_a, hsv[:, 2 * qi + 1],
                    op0=mybir.AluOpType.mult, op1=mybir.AluOpType.add)
                D[q] = DQ
                return D

            def prep_group(raw):
                D = {}
                for qi, q in enumerate(("hl", "lh", "hh")):
                    prep_quad(raw, D, qi, q)
                return raw, D

            def emit_stage1(state, ci, si):
                """Stage 1 for local slice ci: 4 matmuls into one PSUM bank
                then one full-width psum->sbuf copy."""
                (TL, _HN, _HS), D = state
                cs = slice(ci * 128, (ci + 1) * 128)
                P = pp.tile([128, 512], F32, tag="P")
                mm(P[:, 0:256], TL[:, cs], S_TL,
                   start=True, stop=False, skip_group_check=True)
                mm(P[:, 0:256], D["lh"][:, cs], S_E1,
                   start=False, stop=False, skip_group_check=True)
                mm(P[:, 256:512], D["hl"][:, cs], S_E0,
                   start=False, stop=False, skip_group_check=True)
                mm(P[:, 256:512], D["hh"][:, cs], S_E1,
                   start=False, stop=True, skip_group_check=True)
                t = ttp.tile([128, 512], BF16, tag="tts")
                if si % 2 == 0:
                    nc.vector.tensor_copy(t[:], P[:])
                else:
                    nc.scalar.copy(t[:], P[:])
                return t

            yb_state = {}

            def emit_stage2(t, si):
                """Stage 2 for global slice si: 4 matmuls; per pair one bf16
                staging copy; per 4 slices one output DMA."""
                if si % 2 == 0:
                    ypp_new = ppy.tile([128, 1024], F32, tag="YPP")
                    yb_state["ypp"] = ypp_new
                YPP = yb_state["ypp"]
                b = (si % 2) * 512
                mm(YPP[:, b:b + 256], t[:, 0:128], R_lo,
                   start=True, stop=False, skip_group_check=True)
                mm(YPP[:, b:b + 256], t[:, 256:384], R_hi,
                   start=False, stop=False, skip_group_check=True)
                mm(YPP[:, b + 256:b + 512], t[:, 128:256], R_lo,
                   start=False, stop=False, skip_group_check=True)
                mm(YPP[:, b + 256:b + 512], t[:, 384:512], R_hi,
                   start=False, stop=True, skip_group_check=True)
                if si % 4 == 1:
                    yb_new = yp.tile([128, 2048], BF16, tag="YB")
                    yb_state["yb"] = yb_new
                if si % 2 == 1:
                    YB = yb_state["yb"]
                    half = ((si // 2) % 2) * 1024
                    nc.scalar.copy(YB[:, half:half + 1024], YPP[:])
                    if si % 4 == 3:
                        c4 = si - 3
                        nc.gpsimd.dma_start(
                            OUT[c4:c4 + 4].rearrange("g p f -> p g f"),
                            YB.rearrange("p (g f) -> p g f", g=4))

            # software pipeline, slice-granular: stage 2 trails stage 1 by
            # two slices; next group's loads at slice 0, its c2q prep
            # spread over slices 3-5.
            state = prep_group(load_group(0))
            nxt_raw = None
            nxt_D = None
            pend = [None, None]
            for si in range(n_ch):
                g, ci = divmod(si, G)
                if ci == 0 and g + 1 < n_groups:
                    nxt_raw = load_group((g + 1) * G)
                    nxt_D = {}
                cur = emit_stage1(state, ci, si)
                if pend[0] is not None:
                    emit_stage2(pend[0], si - 2)
                pend = [pend[1], cur]
                if g + 1 < n_groups and 3 <= ci <= 5:
                    qi = ci - 3
                    prep_quad(nxt_raw, nxt_D, qi, ("hl", "lh", "hh")[qi])
                if ci == G - 1 and g + 1 < n_groups:
                    state = (nxt_raw, nxt_D)
            emit_stage2(pend[0], n_ch - 2)
            emit_stage2(pend[1], n_ch - 1)

    nc.compile()
    return nc


# ---------------- host wrapper: shard, run on 8 cores, gather ----------------

_CACHED = {}


def _get_compiled():
    if "nc" not in _CACHED:
        _CACHED["nc"] = build_kernel(n_ch=64, G=8, n_cores=8)
        _CACHED["stats"] = build_statics()
    return _CACHED["nc"], _CACHED["stats"]


def _pack_inputs(Yl_b, Yhr_b, Yhi_b):
    """Per-batch host repack into partition-major bf16 layouts."""
    t = Yl_b.transpose(1, 0, 2)                            # [p, ch, c]
    yl2 = np.concatenate([t[..., 0::2], t[..., 1::2]], axis=2)  # [p,ch,128]
    stn = np.stack([Yhr_b[:, BANDS], Yhi_b[:, BANDS]], axis=3)  # [ch,6,r,2,c]
    yhn = stn.transpose(2, 3, 1, 0, 4).reshape(128, 6, Yl_b.shape[0], 64)
    sts = np.stack([Yhi_b[:, BANDS], Yhr_b[:, BANDS]], axis=3)
    yhs = sts.transpose(2, 3, 1, 0, 4).reshape(128, 6, Yl_b.shape[0], 64)
    return (np.ascontiguousarray(yl2.astype(BF)),
            np.ascontiguousarray(yhn.astype(BF)),
            np.ascontiguousarray(yhs.astype(BF)))


def _make_in_maps(Yl, Yhr, Yhi, stats):
    STAT1, STAT2, SIGNS = stats
    in_maps = []
    for b in range(Yl.shape[0]):
        yl2, yhn, yhs = _pack_inputs(Yl[b], Yhr[b], Yhi[b])
        in_maps.append({"YL2": yl2, "YHN": yhn, "YHS": yhs, "STAT1": STAT1,
                        "STAT2": STAT2, "SIGNS": SIGNS})
    return in_maps


def _unpack_out(res_y):
    # [ch, p(128), (h c)] bf16 -> [ch, 256, 256] f32, row = h*128 + p
    y = np.asarray(res_y).astype(np.float32).reshape(64, 128, 2, 256)
    return y.transpose(0, 2, 1, 3).reshape(64, 256, 256)


def kernel(Yl, Yhr, Yhi):
    """Inverse DTCWT (qshift) level. Yl (8,64,128,128) f32,
    Yhr/Yhi (8,64,6,64,64) f32 -> (8,64,256,256) f32.
    Data-parallel over the batch dim: one batch element per NeuronCore."""
    from concourse.bass_utils import run_bass_kernel_spmd

    Yl = np.asarray(Yl, dtype=np.float32)
    Yhr = np.asarray(Yhr, dtype=np.float32)
    Yhi = np.asarray(Yhi, dtype=np.float32)
    B = Yl.shape[0]
    assert B == 8, f"expected batch 8, got {B}"

    nc, stats = _get_compiled()
    in_maps = _make_in_maps(Yl, Yhr, Yhi, stats)
    res = run_bass_kernel_spmd(nc, in_maps, core_ids=list(range(B)))
    out = np.stack([_unpack_out(res.results[b]["Y"]) for b in range(B)])
    return out.astype(np.float32)


# revision 20
# speedup vs baseline: 1.2373x; 1.0023x over previous
"""DTCWT inverse (qshift, single level) as a Bass/Tile kernel for TRN2.

Per-core computation, per channel slice:  Y = Ccat @ Xcat @ Rcat
with Xcat = [[Yl, hl], [lh, hh]] (c2q quadrants), Ccat/Rcat static banded
synthesis matrices, realized as two matmul stages (data stationary, static
matrices moving).

Key structural choices:
  - bf16 operands end-to-end (PSUM accumulation stays f32); host casts the
    bf16 output back to f32.
  - host repacks all inputs into partition-major layouts (>=1KB contiguous
    DMA lines) and stores YH twice: once re/im-interleaved on partitions
    and once im/re-interleaved.  With the swapped copy, the odd-column c2q
    rows come out UNswapped, so even and odd quad columns share the same
    moving static, and the per-slice stationary is one contiguous
    [E-cols | O-cols] block of 128.
  - stage 1 is then 4 matmuls per slice (M=128, N=256) into one PSUM bank
    whose layout equals what stage 2 needs: a single full-width [128,512]
    psum->sbuf copy per slice (no partition remaps).
  - stage 2: 4 matmuls per slice; output staged bf16 and DMA'd per 4
    slices with 1KB contiguous lines.
"""
import numpy as np
import ml_dtypes

import concourse.bacc as bacc
import concourse.tile as tile
from concourse import mybir

F32 = mybir.dt.float32
BF16 = mybir.dt.bfloat16
BF = ml_dtypes.bfloat16

# ---------------- host-side static matrix construction ----------------

_H0A = np.array([0.0351638365171441, 0.0, -0.0883294244510729,
                 0.233890320607236, 0.760272369066126, 0.587518297723561,
                 0.0, -0.114301837144249, 0.0, 0.0], dtype=np.float64)
_H0B = _H0A[::-1].copy()
_ALT = (-1.0) ** np.arange(10)
_H1A = _H0B * _ALT
_H1B = _H1A[::-1].copy()
G0A, G0B, G1A, G1B = _H0B, _H0A, _H1B, _H1A

PI = np.concatenate([np.arange(0, 128, 2), np.arange(1, 128, 2)])
BANDS = [2, 3, 0, 5, 1, 4]  # (hl.b1, hl.b2, lh.b1, lh.b2, hh.b1, hh.b2)


def _reflect(x, minx, maxx):
    x = np.asarray(x, dtype=np.float64)
    rng = maxx - minx
    rng2 = 2.0 * rng
    mod = np.fmod(x - minx, rng2)
    normed = np.where(mod < 0, mod + rng2, mod)
    return (np.where(normed >= rng, rng2 - normed, normed) + minx).astype(np.int64)


def _colifilt_matrix(ha, hb, r=128):
    """C (2r x r) with colifilt(X) = C @ X."""
    m = ha.shape[0]
    m2 = m // 2
    xe = _reflect(np.arange(-m2, r + m2), -0.5, r - 0.5)
    t = np.arange(2, r + m - 1, 2)
    if float(np.sum(ha * hb)) > 0:
        ta, tb = t, t - 1
    else:
        ta, tb = t - 1, t
    r2 = r // 2
    hao, hae = ha[0::2], ha[1::2]
    hbo, hbe = hb[0::2], hb[1::2]

    def vconv_mat(sel_idx, h):
        hf = h[::-1]
        M = np.zeros((r2, r), dtype=np.float64)
        for i in range(r2):
            for k in range(m2):
                M[i, sel_idx[i + k]] += hf[k]
        return M

    C = np.zeros((2 * r, r), dtype=np.float64)
    C[0::4] = vconv_mat(xe[tb], hao)
    C[1::4] = vconv_mat(xe[ta], hbo)
    C[2::4] = vconv_mat(xe[tb], hae)
    C[3::4] = vconv_mat(xe[ta], hbe)
    return C


def build_statics():
    """STAT1 (128 x 768) = [S_TL | S_E0 | S_E1]; STAT2 (128 x 512) =
    [R_lo | R_hi] rows pi-permuted; SIGNS (128 x 1) c2q signs."""
    C0 = _colifilt_matrix(G0B, G0A)
    C1 = _colifilt_matrix(G1B, G1A)
    s = 1.0 / np.sqrt(2.0)
    STAT1 = np.concatenate([C0.T, (s * C0).T, (s * C1).T],
                           axis=1).astype(np.float32)
    STAT2 = np.concatenate([C0.T[PI], C1.T[PI]], axis=1).astype(np.float32)
    SIGNS = np.where(np.arange(128) % 2 == 0, 1.0, -1.0)[:, None]
    return (np.ascontiguousarray(STAT1.astype(BF)),
            np.ascontiguousarray(STAT2.astype(BF)),
            np.ascontiguousarray(SIGNS.astype(BF)))


# ---------------- device kernel ----------------


def build_kernel(n_ch=64, G=8, n_cores=8):
    """Per-core Bass module; processes n_ch channel slices in groups of G."""
    nc = bacc.Bacc("TRN2", target_bir_lowering=False, debug=False,
                   num_devices=n_cores)
    YL2 = nc.dram_tensor("YL2", [128, n_ch, 128], BF16,
                         kind="ExternalInput").ap()
    # bands 0-5: re/im partition interleave; bands 6-11: im/re (swapped)
    YH = nc.dram_tensor("YH", [128, 12, n_ch, 64], BF16,
                        kind="ExternalInput").ap()
    STC = nc.dram_tensor("STC", [128, 1282], BF16, kind="ExternalInput").ap()
    OUT = nc.dram_tensor("Y", [n_ch, 128, 512], BF16,
                         kind="ExternalOutput").ap()

    assert n_ch % G == 0 and G % 2 == 0
    n_groups = n_ch // G

    with tile.TileContext(nc) as tc:
        with (
            tc.tile_pool(name="const", bufs=1) as const,
            tc.tile_pool(name="inp", bufs=3) as inp,
            tc.tile_pool(name="quad", bufs=3) as quad,
            tc.tile_pool(name="tt", bufs=4) as ttp,
            tc.tile_pool(name="yout", bufs=2) as yp,
            tc.tile_pool(name="psum", bufs=4, space="PSUM") as pp,
            tc.tile_pool(name="psumy", bufs=2, space="PSUM") as ppy,
        ):
            stc = const.tile([128, 1282], BF16)
            nc.sync.dma_start(stc[:], STC[:])

            S_TL = stc[:, 0:256]
            S_E0 = stc[:, 256:512]
            S_E1 = stc[:, 512:768]
            R_lo = stc[:, 768:1024]
            R_hi = stc[:, 1024:1280]
            s_a = stc[:, 1280:1281]
            mm = nc.tensor.matmul

            def load_group(g0):
                TL = inp.tile([128, 128 * G], BF16, tag="TL")
                nc.sync.dma_start(
                    TL.rearrange("p (g x) -> p g x", g=G),
                    YL2[:, g0:g0 + G])
                HT = inp.tile([128, 768 * G], BF16, tag="HT")
                nc.sync.dma_start(
                    HT.rearrange("p (k g c) -> p k g c", k=12, g=G),
                    YH[:, :, g0:g0 + G])
                return TL, HT

            def prep_quad(raw, D, qi, q):
                """c2q for one quadrant of a group: E from the natural
                interleave, O (unswapped) from the swapped interleave."""
                _TL, HT = raw
                hv = HT.rearrange("p (k g c) -> p k g c", k=12, g=G)
                DQ = quad.tile([128, 128 * G], BF16, tag=f"DQ_{q}")
                dv = DQ.rearrange("p (g eo c) -> p g eo c", g=G, eo=2)
                nc.vector.scalar_tensor_tensor(
                    dv[:, :, 0], hv[:, 2 * qi + 1], s_a, hv[:, 2 * qi],
                    op0=mybir.AluOpType.mult, op1=mybir.AluOpType.add)
                nc.vector.scalar_tensor_tensor(
                    dv[:, :, 1], hv[:, 6 + 2 * qi], s_a, hv[:, 6 + 2 * qi + 1],
                    op0=mybir.AluOpType.mult, op1=mybir.AluOpType.add)
                D[q] = DQ
                return D

            def prep_group(raw):
                D = {}
                for qi, q in enumerate(("hl", "lh", "hh")):
                    prep_quad(raw, D, qi, q)
                return raw, D

            def emit_stage1(state, ci, si):
                """Stage 1 for local slice ci: 4 matmuls into one PSUM bank
                then one full-width psum->sbuf copy."""
                (TL, _HT), D = state
                cs = slice(ci * 128, (ci + 1) * 128)
                P = pp.tile([128, 512], F32, tag="P")
                mm(P[:, 0:256], TL[:, cs], S_TL,
                   start=True, stop=False, skip_group_check=True)
                mm(P[:, 0:256], D["lh"][:, cs], S_E1,
                   start=False, stop=False, skip_group_check=True)
                mm(P[:, 256:512], D["hl"][:, cs], S_E0,
                   start=False, stop=False, skip_group_check=True)
                mm(P[:, 256:512], D["hh"][:, cs], S_E1,
                   start=False, stop=True, skip_group_check=True)
                t = ttp.tile([128, 512], BF16, tag="tts")
                if si % 2 == 0:
                    nc.vector.tensor_copy(t[:], P[:])
                else:
                    nc.scalar.copy(t[:], P[:])
                return t

            yb_state = {}

            def emit_stage2(t, si):
                """Stage 2 for global slice si: 4 matmuls; per pair one bf16
                staging copy; per 4 slices one output DMA."""
                if si % 2 == 0:
                    ypp_new = ppy.tile([128, 1024], F32, tag="YPP")
                    yb_state["ypp"] = ypp_new
                YPP = yb_state["ypp"]
                b = (si % 2) * 512
                mm(YPP[:, b:b + 256], t[:, 0:128], R_lo,
                   start=True, stop=False, skip_group_check=True)
                mm(YPP[:, b:b + 256], t[:, 256:384], R_hi,
                   start=False, stop=False, skip_group_check=True)
                mm(YPP[:, b + 256:b + 512], t[:, 128:256], R_lo,
                   start=False, stop=False, skip_group_check=True)
                mm(YPP[:, b + 256:b + 512], t[:, 384:512], R_hi,
                   start=False, stop=True, skip_group_check=True)
                if si % 4 == 1:
                    yb_new = yp.tile([128, 2048], BF16, tag="YB")
                    yb_state["yb"] = yb_new
                if si % 2 == 1:
                    YB = yb_state["yb"]
                    half = ((si // 2) % 2) * 1024
                    nc.scalar.copy(YB[:, half:half + 1024], YPP[:])
                    if si % 4 == 3:
                        c4 = si - 3
                        nc.gpsimd.dma_start(
                            OUT[c4:c4 + 4].rearrange("g p f -> p g f"),
                            YB.rearrange("p (g f) -> p g f", g=4))

            # software pipeline, slice-granular: stage 2 trails stage 1 by
            # two slices; next group's loads at slice 0, its c2q prep
            # spread over slices 3-5.
            state = prep_group(load_group(0))
            nxt_raw = None
            nxt_D = None
            pend = [None, None]
            for si in range(n_ch):
                g, ci = divmod(si, G)
                if ci == 0 and g + 1 < n_groups:
                    nxt_raw = load_group((g + 1) * G)
                    nxt_D = {}
                cur = emit_stage1(state, ci, si)
                if pend[0] is not None:
                    emit_stage2(pend[0], si - 2)
                pend = [pend[1], cur]
                if g + 1 < n_groups and 3 <= ci <= 5:
                    qi = ci - 3
                    prep_quad(nxt_raw, nxt_D, qi, ("hl", "lh", "hh")[qi])
                if ci == G - 1 and g + 1 < n_groups:
                    state = (nxt_raw, nxt_D)
            emit_stage2(pend[0], n_ch - 2)
            emit_stage2(pend[1], n_ch - 1)

    nc.compile()
    return nc


# ---------------- host wrapper: shard, run on 8 cores, gather ----------------

_CACHED = {}


def _get_compiled():
    if "nc" not in _CACHED:
        _CACHED["nc"] = build_kernel(n_ch=64, G=8, n_cores=8)
        _CACHED["stats"] = build_statics()
    return _CACHED["nc"], _CACHED["stats"]


def _pack_inputs(Yl_b, Yhr_b, Yhi_b):
    """Per-batch host repack into partition-major bf16 layouts."""
    t = Yl_b.transpose(1, 0, 2)                            # [p, ch, c]
    yl2 = np.concatenate([t[..., 0::2], t[..., 1::2]], axis=2)  # [p,ch,128]
    stn = np.stack([Yhr_b[:, BANDS], Yhi_b[:, BANDS]], axis=3)  # [ch,6,r,2,c]
    yhn = stn.transpose(2, 3, 1, 0, 4).reshape(128, 6, Yl_b.shape[0], 64)
    sts = np.stack([Yhi_b[:, BANDS], Yhr_b[:, BANDS]], axis=3)
    yhs = sts.transpose(2, 3, 1, 0, 4).reshape(128, 6, Yl_b.shape[0], 64)
    yh = np.concatenate([yhn, yhs], axis=1)                # [p, 12, ch, 64]
    return (np.ascontiguousarray(yl2.astype(BF)),
            np.ascontiguousarray(yh.astype(BF)))


def _make_in_maps(Yl, Yhr, Yhi, stats):
    STAT1, STAT2, SIGNS = stats
    sgn2 = np.concatenate([SIGNS, np.zeros((128, 1), SIGNS.dtype)], axis=1)
    stc = np.ascontiguousarray(
        np.concatenate([STAT1, STAT2, sgn2], axis=1))      # [128, 1282]
    in_maps = []
    for b in range(Yl.shape[0]):
        yl2, yh = _pack_inputs(Yl[b], Yhr[b], Yhi[b])
        in_maps.append({"YL2": yl2, "YH": yh, "STC": stc})
    return in_maps


def _unpack_out(res_y):
    # [ch, p(128), (h c)] bf16 -> [ch, 256, 256] f32, row = h*128 + p
    y = np.asarray(res_y).astype(np.float32).reshape(64, 128, 2, 256)
    return y.transpose(0, 2, 1, 3).reshape(64, 256, 256)


def kernel(Yl, Yhr, Yhi):
    """Inverse DTCWT (qshift) level. Yl (8,64,128,128) f32,
    Yhr/Yhi (8,64,6,64,64) f32 -> (8,64,256,256) f32.
    Data-parallel over the batch dim: one batch element per NeuronCore."""
    from concourse.bass_utils import run_bass_kernel_spmd

    Yl = np.asarray(Yl, dtype=np.float32)
    Yhr = np.asarray(Yhr, dtype=np.float32)
    Yhi = np.asarray(Yhi, dtype=np.float32)
    B = Yl.shape[0]
    assert B == 8, f"expected batch 8, got {B}"

    nc, stats = _get_compiled()
    in_maps = _make_in_maps(Yl, Yhr, Yhi, stats)
    res = run_bass_kernel_spmd(nc, in_maps, core_ids=list(range(B)))
    out = np.stack([_unpack_out(res.results[b]["Y"]) for b in range(B)])
    return out.astype(np.float32)


# revision 25
# speedup vs baseline: 1.2951x; 1.0467x over previous
"""DTCWT inverse (qshift, single level) as a Bass/Tile kernel for TRN2.

Per-core computation, per channel slice:  Y = Ccat @ Xcat @ Rcat
with Xcat = [[Yl, hl], [lh, hh]] (c2q quadrants), Ccat/Rcat static banded
synthesis matrices, realized as two matmul stages (data stationary, static
matrices moving).

Key structural choices:
  - bf16 operands end-to-end (PSUM accumulation stays f32); host casts the
    bf16 output back to f32.
  - host repacks all inputs into partition-major layouts (>=1KB contiguous
    DMA lines) and stores YH twice: once re/im-interleaved on partitions
    and once im/re-interleaved.  With the swapped copy, the odd-column c2q
    rows come out UNswapped, so even and odd quad columns share the same
    moving static, and the per-slice stationary is one contiguous
    [E-cols | O-cols] block of 128.
  - stage 1 is then 4 matmuls per slice (M=128, N=256) into one PSUM bank
    whose layout equals what stage 2 needs: a single full-width [128,512]
    psum->sbuf copy per slice (no partition remaps).
  - stage 2: 4 matmuls per slice; output staged bf16 and DMA'd per 4
    slices with 1KB contiguous lines.
"""
import numpy as np
import ml_dtypes

import concourse.bacc as bacc
import concourse.tile as tile
from concourse import mybir

F32 = mybir.dt.float32
BF16 = mybir.dt.bfloat16
BF = ml_dtypes.bfloat16

# ---------------- host-side static matrix construction ----------------

_H0A = np.array([0.0351638365171441, 0.0, -0.0883294244510729,
                 0.233890320607236, 0.760272369066126, 0.587518297723561,
                 0.0, -0.114301837144249, 0.0, 0.0], dtype=np.float64)
_H0B = _H0A[::-1].copy()
_ALT = (-1.0) ** np.arange(10)
_H1A = _H0B * _ALT
_H1B = _H1A[::-1].copy()
G0A, G0B, G1A, G1B = _H0B, _H0A, _H1B, _H1A

PI = np.concatenate([np.arange(0, 128, 2), np.arange(1, 128, 2)])
BANDS = [2, 3, 0, 5, 1, 4]  # (hl.b1, hl.b2, lh.b1, lh.b2, hh.b1, hh.b2)


def _reflect(x, minx, maxx):
    x = np.asarray(x, dtype=np.float64)
    rng = maxx - minx
    rng2 = 2.0 * rng
    mod = np.fmod(x - minx, rng2)
    normed = np.where(mod < 0, mod + rng2, mod)
    return (np.where(normed >= rng, rng2 - normed, normed) + minx).astype(np.int64)


def _colifilt_matrix(ha, hb, r=128):
    """C (2r x r) with colifilt(X) = C @ X."""
    m = ha.shape[0]
    m2 = m // 2
    xe = _reflect(np.arange(-m2, r + m2), -0.5, r - 0.5)
    t = np.arange(2, r + m - 1, 2)
    if float(np.sum(ha * hb)) > 0:
        ta, tb = t, t - 1
    else:
        ta, tb = t - 1, t
    r2 = r // 2
    hao, hae = ha[0::2], ha[1::2]
    hbo, hbe = hb[0::2], hb[1::2]

    def vconv_mat(sel_idx, h):
        hf = h[::-1]
        M = np.zeros((r2, r), dtype=np.float64)
        for i in range(r2):
            for k in range(m2):
                M[i, sel_idx[i + k]] += hf[k]
        return M

    C = np.zeros((2 * r, r), dtype=np.float64)
    C[0::4] = vconv_mat(xe[tb], hao)
    C[1::4] = vconv_mat(xe[ta], hbo)
    C[2::4] = vconv_mat(xe[tb], hae)
    C[3::4] = vconv_mat(xe[ta], hbe)
    return C


def build_statics():
    """STAT1 (128 x 768) = [S_TL | S_E0 | S_E1]; STAT2 (128 x 512) =
    [R_lo | R_hi] rows pi-permuted; SIGNS (128 x 1) c2q signs."""
    C0 = _colifilt_matrix(G0B, G0A)
    C1 = _colifilt_matrix(G1B, G1A)
    s = 1.0 / np.sqrt(2.0)
    STAT1 = np.concatenate([C0.T, (s * C0).T, (s * C1).T],
                           axis=1).astype(np.float32)
    STAT2 = np.concatenate([C0.T[PI], C1.T[PI]], axis=1).astype(np.float32)
    SIGNS = np.where(np.arange(128) % 2 == 0, 1.0, -1.0)[:, None]
    return (np.ascontiguousarray(STAT1.astype(BF)),
            np.ascontiguousarray(STAT2.astype(BF)),
            np.ascontiguousarray(SIGNS.astype(BF)))


# ---------------- device kernel ----------------


def build_kernel(n_ch=64, G=8, n_cores=8):
    """Per-core Bass module; processes n_ch channel slices in groups of G."""
    nc = bacc.Bacc("TRN2", target_bir_lowering=False, debug=False,
                   num_devices=n_cores)
    # flat per-group-contiguous layouts: one DMA descriptor per partition
    YL2 = nc.dram_tensor("YL2", [128, n_ch * 128], BF16,
                         kind="ExternalInput").ap()
    # per group block of 6144: (band 12, ch 8, col 64); bands 0-5 re/im
    # partition interleave, bands 6-11 im/re (swapped)
    YH = nc.dram_tensor("YH", [128, (n_ch // G) * 768 * G], BF16,
                        kind="ExternalInput").ap()
    STC = nc.dram_tensor("STC", [128, 1282], BF16, kind="ExternalInput").ap()
    OUT = nc.dram_tensor("Y", [128, n_ch * 512], BF16,
                         kind="ExternalOutput").ap()

    assert n_ch % G == 0 and G % 2 == 0
    n_groups = n_ch // G

    with tile.TileContext(nc) as tc:
        with (
            tc.tile_pool(name="const", bufs=1) as const,
            tc.tile_pool(name="inp", bufs=3) as inp,
            tc.tile_pool(name="quad", bufs=3) as quad,
            tc.tile_pool(name="tt", bufs=4) as ttp,
            tc.tile_pool(name="yout", bufs=2) as yp,
            tc.tile_pool(name="psum", bufs=4, space="PSUM") as pp,
            tc.tile_pool(name="psumy", bufs=2, space="PSUM") as ppy,
        ):
            stc = const.tile([128, 1282], BF16)
            nc.sync.dma_start(stc[:], STC[:])

            S_TL = stc[:, 0:256]
            S_E0 = stc[:, 256:512]
            S_E1 = stc[:, 512:768]
            R_lo = stc[:, 768:1024]
            R_hi = stc[:, 1024:1280]
            s_a = stc[:, 1280:1281]
            mm = nc.tensor.matmul

            def load_group(g0):
                TL = inp.tile([128, 128 * G], BF16, tag="TL")
                nc.sync.dma_start(TL[:], YL2[:, g0 * 128:(g0 + G) * 128])
                HT = inp.tile([128, 768 * G], BF16, tag="HT")
                nc.sync.dma_start(HT[:], YH[:, g0 * 768:(g0 + G) * 768])
                return TL, HT

            def prep_quad(raw, D, qi, q):
                """c2q for one quadrant of a group: E from the natural
                interleave, O (unswapped) from the swapped interleave."""
                _TL, HT = raw
                hv = HT.rearrange("p (k g c) -> p k g c", k=12, g=G)
                DQ = quad.tile([128, 128 * G], BF16, tag=f"DQ_{q}")
                dv = DQ.rearrange("p (g eo c) -> p g eo c", g=G, eo=2)
                nc.vector.scalar_tensor_tensor(
                    dv[:, :, 0], hv[:, 2 * qi + 1], s_a, hv[:, 2 * qi],
                    op0=mybir.AluOpType.mult, op1=mybir.AluOpType.add)
                nc.vector.scalar_tensor_tensor(
                    dv[:, :, 1], hv[:, 6 + 2 * qi], s_a, hv[:, 6 + 2 * qi + 1],
                    op0=mybir.AluOpType.mult, op1=mybir.AluOpType.add)
                D[q] = DQ
                return D

            def prep_group(raw):
                D = {}
                for qi, q in enumerate(("hl", "lh", "hh")):
                    prep_quad(raw, D, qi, q)
                return raw, D

            def emit_stage1(state, ci, si):
                """Stage 1 for local slice ci: 4 matmuls into one PSUM bank
                then one full-width psum->sbuf copy."""
                (TL, _HT), D = state
                cs = slice(ci * 128, (ci + 1) * 128)
                P = pp.tile([128, 512], F32, tag="P")
                mm(P[:, 0:256], TL[:, cs], S_TL,
                   start=True, stop=False, skip_group_check=True)
                mm(P[:, 0:256], D["lh"][:, cs], S_E1,
                   start=False, stop=False, skip_group_check=True)
                mm(P[:, 256:512], D["hl"][:, cs], S_E0,
                   start=False, stop=False, skip_group_check=True)
                mm(P[:, 256:512], D["hh"][:, cs], S_E1,
                   start=False, stop=True, skip_group_check=True)
                t = ttp.tile([128, 512], BF16, tag="tts")
                if si % 2 == 0:
                    nc.vector.tensor_copy(t[:], P[:])
                else:
                    nc.scalar.copy(t[:], P[:])
                return t

            yb_state = {}

            def emit_stage2(t, si):
                """Stage 2 for global slice si: 4 matmuls; per pair one bf16
                staging copy; per 4 slices one output DMA."""
                if si % 2 == 0:
                    ypp_new = ppy.tile([128, 1024], F32, tag="YPP")
                    yb_state["ypp"] = ypp_new
                YPP = yb_state["ypp"]
                b = (si % 2) * 512
                mm(YPP[:, b:b + 256], t[:, 0:128], R_lo,
                   start=True, stop=False, skip_group_check=True)
                mm(YPP[:, b:b + 256], t[:, 256:384], R_hi,
                   start=False, stop=False, skip_group_check=True)
                mm(YPP[:, b + 256:b + 512], t[:, 128:256], R_lo,
                   start=False, stop=False, skip_group_check=True)
                mm(YPP[:, b + 256:b + 512], t[:, 384:512], R_hi,
                   start=False, stop=True, skip_group_check=True)
                if si % 4 == 1:
                    yb_new = yp.tile([128, 2048], BF16, tag="YB")
                    yb_state["yb"] = yb_new
                if si % 2 == 1:
                    YB = yb_state["yb"]
                    half = ((si // 2) % 2) * 1024
                    nc.scalar.copy(YB[:, half:half + 1024], YPP[:])
                    if si % 4 == 3:
                        c4 = si - 3
                        nc.gpsimd.dma_start(
                            OUT[:, c4 * 512:(c4 + 4) * 512], YB[:])

            # software pipeline, slice-granular: stage 2 trails stage 1 by
            # two slices; next group's loads at slice 0, its c2q prep
            # spread over slices 3-5.
            state = prep_group(load_group(0))
            nxt_raw = None
            nxt_D = None
            pend = [None, None]
            for si in range(n_ch):
                g, ci = divmod(si, G)
                if ci == 0 and g + 1 < n_groups:
                    nxt_raw = load_group((g + 1) * G)
                    nxt_D = {}
                cur = emit_stage1(state, ci, si)
                if pend[0] is not None:
                    emit_stage2(pend[0], si - 2)
                pend = [pend[1], cur]
                if g + 1 < n_groups and 3 <= ci <= 5:
                    qi = ci - 3
                    prep_quad(nxt_raw, nxt_D, qi, ("hl", "lh", "hh")[qi])
                if ci == G - 1 and g + 1 < n_groups:
                    state = (nxt_raw, nxt_D)
            emit_stage2(pend[0], n_ch - 2)
            emit_stage2(pend[1], n_ch - 1)

    nc.compile()
    return nc


# ---------------- host wrapper: shard, run on 8 cores, gather ----------------

_CACHED = {}


def _get_compiled():
    if "nc" not in _CACHED:
        _CACHED["nc"] = build_kernel(n_ch=64, G=8, n_cores=8)
        _CACHED["stats"] = build_statics()
    return _CACHED["nc"], _CACHED["stats"]


def _pack_inputs(Yl_b, Yhr_b, Yhi_b):
    """Per-batch host repack into partition-major bf16 layouts."""
    t = Yl_b.transpose(1, 0, 2)                            # [p, ch, c]
    yl2 = np.concatenate([t[..., 0::2], t[..., 1::2]], axis=2)  # [p,ch,128]
    stn = np.stack([Yhr_b[:, BANDS], Yhi_b[:, BANDS]], axis=3)  # [ch,6,r,2,c]
    yhn = stn.transpose(2, 3, 1, 0, 4).reshape(128, 6, Yl_b.shape[0], 64)
    sts = np.stack([Yhi_b[:, BANDS], Yhr_b[:, BANDS]], axis=3)
    yhs = sts.transpose(2, 3, 1, 0, 4).reshape(128, 6, Yl_b.shape[0], 64)
    yh = np.concatenate([yhn, yhs], axis=1)                # [p, 12, ch, 64]
    # per-group-contiguous flat blocks: (group, band, ch-in-group, col)
    yhf = yh.reshape(128, 12, 8, 8, 64).transpose(0, 2, 1, 3, 4)
    return (np.ascontiguousarray(yl2.astype(BF)).reshape(128, -1),
            np.ascontiguousarray(yhf.astype(BF)).reshape(128, -1))


def _make_in_maps(Yl, Yhr, Yhi, stats):
    STAT1, STAT2, SIGNS = stats
    sgn2 = np.concatenate([SIGNS, np.zeros((128, 1), SIGNS.dtype)], axis=1)
    stc = np.ascontiguousarray(
        np.concatenate([STAT1, STAT2, sgn2], axis=1))      # [128, 1282]
    in_maps = []
    for b in range(Yl.shape[0]):
        yl2, yh = _pack_inputs(Yl[b], Yhr[b], Yhi[b])
        in_maps.append({"YL2": yl2, "YH": yh, "STC": stc})
    return in_maps


def _unpack_out(res_y):
    # [p(128), ch*(h c)] bf16 -> [ch, 256, 256] f32, row = h*128 + p
    y = np.asarray(res_y).astype(np.float32).reshape(128, 64, 2, 256)
    return y.transpose(1, 2, 0, 3).reshape(64, 256, 256)


def kernel(Yl, Yhr, Yhi):
    """Inverse DTCWT (qshift) level. Yl (8,64,128,128) f32,
    Yhr/Yhi (8,64,6,64,64) f32 -> (8,64,256,256) f32.
    Data-parallel over the batch dim: one batch element per NeuronCore."""
    from concourse.bass_utils import run_bass_kernel_spmd

    Yl = np.asarray(Yl, dtype=np.float32)
    Yhr = np.asarray(Yhr, dtype=np.float32)
    Yhi = np.asarray(Yhi, dtype=np.float32)
    B = Yl.shape[0]
    assert B == 8, f"expected batch 8, got {B}"

    nc, stats = _get_compiled()
    in_maps = _make_in_maps(Yl, Yhr, Yhi, stats)
    res = run_bass_kernel_spmd(nc, in_maps, core_ids=list(range(B)))
    out = np.stack([_unpack_out(res.results[b]["Y"]) for b in range(B)])
    return out.astype(np.float32)
